# revision 1
# baseline (speedup 1.0000x reference)
"""Multi-head attention (B=128, T=256, D=512, H=8, HD=64) on 8 TRN2 NeuronCores.

Data-parallel over batch (16 batches per core), weights replicated.
Per-core Bass/Tile kernel in "transposed" space:

  xT[d, t]      <- PE-transpose of x[t, d]                 (per batch)
  QT/KT[hd, t]  <- Wq/Wk-pair.T @ xT (f32r, head pairs packed, M=128)
  V'[s, *]      <- xT-chunk.T @ Wv; per head the 128 lhsT columns are
                   [V_h | ones], so V'.T @ expT yields oT (rows 0-63)
                   AND the softmax denominator broadcast (rows 64-127)
                   in ONE accumulation group - no separate colsum matmul.
  scT[s, t]     <- KT-slice.T @ QT  == scores^T            (per head)
  expT          <- exp(0.125 * scT) (ACT); causal mask via gpsimd
                   affine_select on the two triangular 128x128 blocks only;
                   the all-dead quarter is pre-zeroed once in a ring buffer.
  catT          <- oT * recip(denom)   (DVE)
  out[t, :]     <- catT-chunk.T @ Wo + bo (bias pre-broadcast via PE)

All f32r matmul operands are produced by compute ops (DVE/ACT copy,
activation, affine_select) to satisfy the BIR verifier rounding rule.
Batch b+1's load/transpose/projection work is interleaved into batch b's
head loop to keep the PE fed.
"""
from collections import deque
from contextlib import ExitStack

import numpy as np

import jax
import concourse.bass as bass
import concourse.mybir as mybir
import concourse.tile as tile
from concourse import bacc
from concourse.masks import make_identity

F32 = mybir.dt.float32
F32R = mybir.dt.float32r
EXP = mybir.ActivationFunctionType.Exp

NCORES = 8
B, T, D, H, HD = 128, 256, 512, 8, 64
BL = B // NCORES          # batches per core
NCH = D // 128            # 4 contraction chunks of 128
NPAIR = H // 2            # 4 head pairs
SCALE = float(HD) ** -0.5  # 0.125
NEXP = 6                  # expT ring depth


def _emit(nc):
    x_d = nc.dram_tensor("x", [BL, T, D], F32, kind="ExternalInput")
    wq_d = nc.dram_tensor("Wq", [H, D, HD], F32, kind="ExternalInput")
    wk_d = nc.dram_tensor("Wk", [H, D, HD], F32, kind="ExternalInput")
    wv_d = nc.dram_tensor("Wv", [H, D, HD], F32, kind="ExternalInput")
    wo_d = nc.dram_tensor("Wo", [D, D], F32, kind="ExternalInput")
    bo_d = nc.dram_tensor("bo", [1, D], F32, kind="ExternalInput")
    out_d = nc.dram_tensor("out", [BL, T, D], F32, kind="ExternalOutput")

    with tile.TileContext(nc) as tc:
        with ExitStack() as ctx:
            const = ctx.enter_context(tc.tile_pool(name="const", bufs=1))
            wst = ctx.enter_context(tc.tile_pool(name="wst", bufs=2))
            xp = ctx.enter_context(tc.tile_pool(name="xp", bufs=4))
            xtp = ctx.enter_context(tc.tile_pool(name="xtp", bufs=2))
            qkvp = ctx.enter_context(tc.tile_pool(name="qkvp", bufs=2))
            recp = ctx.enter_context(tc.tile_pool(name="recp", bufs=4))
            osbp = ctx.enter_context(tc.tile_pool(name="osbp", bufs=3))
            # PSUM: 8 banks = big(1) + proj(2) + sc(3) + att(2)
            psum = ctx.enter_context(tc.tile_pool(name="ps", bufs=2, space="PSUM"))

            # ---- constants -------------------------------------------------
            ident = const.tile([128, 128], F32)
            make_identity(nc, ident)
            ones1 = const.tile([1, 128], F32)
            nc.gpsimd.memset(ones1, 1.0)
            ones_blk = const.tile([128, 2, H, HD], F32)
            nc.gpsimd.memset(ones_blk, 1.0)
            zblk = const.tile([128, 128], F32)
            nc.gpsimd.memset(zblk, 0.0)
            bo_sb = const.tile([1, D], F32)
            nc.sync.dma_start(bo_sb, bo_d[:, :])
            bo_ps = psum.tile([128, D], F32, tag="big", bufs=2)
            nc.tensor.matmul(bo_ps, ones1, bo_sb, start=True, stop=True)
            bo_bc = const.tile([128, D], F32)
            nc.vector.tensor_copy(bo_bc, bo_ps)

            # V ring: [s, sc, h, {V|ones}, hd]; ones half preset once
            V_bufs = []
            for i in range(2):
                vb = const.tile([128, 2, H, 2, HD], F32R, name=f"Vbuf{i}")
                nc.vector.tensor_copy(vb[:, :, :, 1, :], ones_blk)
                V_bufs.append(vb)
            # expT ring: [s, sc, t]; dead quarter (sc=1, t<128) pre-zeroed
            expT_bufs = []
            for i in range(NEXP):
                eb = const.tile([128, 2, T], F32R, name=f"expTbuf{i}")
                nc.vector.tensor_copy(eb[:, 1, 0:128], zblk)
                expT_bufs.append(eb)

            # weights loaded later (after batch-0 x DMA is queued) so the
            # first x load isn't stuck behind 16 weight-chunk DMAs
            w_r = {}
            wo_r = None

            def emit_weights():
                nonlocal wo_r
                # on the ACT hwdge queue, parallel to x loads on SP's
                for nm, wd in (("q", wq_d), ("k", wk_d), ("v", wv_d)):
                    stg = wst.tile([128, NCH, D], F32, tag="wstage",
                                   name=f"stg_{nm}")
                    wr = const.tile([128, NCH, D], F32R, name=f"w_{nm}")
                    for c in range(NCH):
                        nc.scalar.dma_start(
                            stg[:, c, :].rearrange("p (h k) -> p h k", h=H),
                            wd[:, c * 128:(c + 1) * 128, :].rearrange(
                                "h p k -> p h k"))
                        nc.vector.tensor_copy(wr[:, c, :], stg[:, c, :])
                    w_r[nm] = wr
                stg_o = wst.tile([128, NCH, D], F32, tag="wstage")
                wo_r = const.tile([128, NCH, D], F32R)
                for c in range(NCH):
                    nc.scalar.dma_start(stg_o[:, c, :],
                                        wo_d[c * 128:(c + 1) * 128, :])
                    nc.vector.tensor_copy(wo_r[:, c, :], stg_o[:, c, :])

            state = {}

            def make_batch_units(b):
                """Closures for batch-b prep: load, 4 transpose units, 6 proj."""
                units = []

                def u_load():
                    xts = []
                    for tci in range(2):
                        x_t = xp.tile([128, D], F32, tag="x",
                                      name=f"x_{b}_{tci}")
                        nc.sync.dma_start(
                            x_t, x_d[b, tci * 128:(tci + 1) * 128, :])
                        xts.append(x_t)
                    xT = xtp.tile([128, NCH, T], F32R, tag="xT", name=f"xT_{b}")
                    state[b] = {"xts": xts, "xT": xT}
                units.append(u_load)

                def u_transpose(c):
                    def f():
                        st = state[b]
                        tp_ps = psum.tile([128, T], F32, tag="big", bufs=2,
                                          name=f"tp_{b}_{c}")
                        for tci in range(2):
                            nc.tensor.transpose(
                                tp_ps[:, tci * 128:(tci + 1) * 128],
                                st["xts"][tci][:, c * 128:(c + 1) * 128],
                                ident)
                        nc.scalar.copy(st["xT"][:, c, :], tp_ps)
                    return f
                units += [u_transpose(c) for c in range(NCH)]

                def u_projqk(nm, j, dst_key, eng):
                    def f():
                        st = state[b]
                        if dst_key not in st:
                            st[dst_key] = qkvp.tile(
                                [128, NPAIR, T], F32R, tag=dst_key,
                                name=f"{dst_key}_{b}")
                        pj = psum.tile([128, 2, T], F32, tag="proj", bufs=1,
                                       name=f"pj_{nm}_{b}_{j}")
                        for p2 in range(2):
                            p = 2 * j + p2
                            for c in range(NCH):
                                nc.tensor.matmul(
                                    pj[:, p2, :],
                                    w_r[nm][:, c, p * 128:(p + 1) * 128],
                                    st["xT"][:, c, :],
                                    start=(c == 0), stop=(c == NCH - 1))
                        if eng == "act":
                            nc.scalar.copy(
                                st[dst_key][:, 2 * j:2 * j + 2, :], pj)
                        else:
                            nc.vector.tensor_copy(
                                st[dst_key][:, 2 * j:2 * j + 2, :], pj)
                    return f
                units += [u_projqk("q", 0, "QT", "dve"),
                          u_projqk("q", 1, "QT", "dve"),
                          u_projqk("k", 0, "KT", "dve"),
                          u_projqk("k", 1, "KT", "dve")]

                def u_projv(sc):
                    def f():
                        st = state[b]
                        vb = V_bufs[b % 2]
                        pj = psum.tile([128, H, HD], F32, tag="proj", bufs=1,
                                       name=f"pj_v_{b}_{sc}")
                        for q in range(2):
                            for c in range(NCH):
                                nc.tensor.matmul(
                                    pj[:, 4 * q:4 * (q + 1), :],
                                    st["xT"][:, c, sc * 128:(sc + 1) * 128],
                                    w_r["v"][:, c, q * 256:(q + 1) * 256],
                                    start=(c == 0), stop=(c == NCH - 1))
                        nc.scalar.copy(vb[:, sc, :, 0, :], pj)
                    return f
                units += [u_projv(0), u_projv(1)]
                return units

            def emit_scores(b, h):
                st = state[b]
                p, hh = divmod(h, 2)
                pb = hh * HD
                sc_ps = psum.tile([128, 2, T], F32, tag="sc", bufs=3,
                                  name=f"sc_{b}_{h}")
                for sc in range(2):
                    nc.tensor.matmul(
                        sc_ps[:, sc, :],
                        st["KT"][pb:pb + HD, p, sc * 128:(sc + 1) * 128],
                        st["QT"][pb:pb + HD, p, :],
                        start=True, stop=True)
                eb = expT_bufs[(b * H + h) % NEXP]
                nc.scalar.activation(eb[:, 0, :], sc_ps[:, 0, :], EXP,
                                     scale=SCALE)
                nc.scalar.activation(eb[:, 1, 128:256], sc_ps[:, 1, 128:256],
                                     EXP, scale=SCALE)
                # causal triangles: keep where t - s >= 0
                nc.gpsimd.affine_select(
                    out=eb[:, 0, 0:128], in_=eb[:, 0, 0:128],
                    compare_op=mybir.AluOpType.is_ge, fill=0.0,
                    base=0, pattern=[[1, 128]], channel_multiplier=-1)
                nc.gpsimd.affine_select(
                    out=eb[:, 1, 128:256], in_=eb[:, 1, 128:256],
                    compare_op=mybir.AluOpType.is_ge, fill=0.0,
                    base=0, pattern=[[1, 128]], channel_multiplier=-1)
                return eb

            def emit_tail(b, h, eb, catT):
                p, hh = divmod(h, 2)
                vb = V_bufs[b % 2]
                ot_ps = psum.tile([128, T], F32, tag="att", name=f"ot_{b}_{h}")
                for sc in range(2):
                    nc.tensor.matmul(ot_ps, vb[:, sc, h, :, :], eb[:, sc, :],
                                     start=(sc == 0), stop=(sc == 1))
                recip = recp.tile([HD, T], F32, tag="rec", name=f"rec_{b}_{h}")
                nc.vector.reciprocal(recip, ot_ps[HD:2 * HD, :])
                nc.vector.tensor_mul(catT[hh * HD:(hh + 1) * HD, p, :],
                                     ot_ps[0:HD, :], recip)

            def mk_outproj(b, catT):
                def one(tci):
                    def f():
                        po = psum.tile([128, D], F32, tag="big", bufs=2,
                                       name=f"po_{b}_{tci}")
                        for c in range(NCH):
                            nc.tensor.matmul(
                                po, catT[:, c, tci * 128:(tci + 1) * 128],
                                wo_r[:, c, :],
                                start=(c == 0), stop=(c == NCH - 1))
                        osb = osbp.tile([128, D], F32, tag="osb",
                                        name=f"osb_{b}_{tci}")
                        nc.vector.tensor_add(osb, po, bo_bc)
                        nc.sync.dma_start(
                            out_d[b, tci * 128:(tci + 1) * 128, :], osb)
                    return f
                return [one(0), one(1)]

            # ---- main pipeline --------------------------------------------
            fillers = deque()
            units0 = make_batch_units(0)
            for u in units0[:5]:
                u()                  # x(0) DMA + transposes first
            units1 = make_batch_units(1)
            units1[0]()              # x(1) DMA also ahead of the weights
            emit_weights()           # weight DMAs on the other queue
            for u in units0[5:]:
                u()                  # batch-0 projections
            pending_out = deque()
            for b in range(BL):
                if b + 1 < BL:
                    fillers.extend(units1[1:] if b == 0
                                   else make_batch_units(b + 1))
                catT = qkvp.tile([128, NPAIR, T], F32R, tag="cat",
                                 name=f"catT_{b}")
                pend = deque()
                for i in range(H + 2):
                    if i < H:
                        pend.append((i, emit_scores(b, i)))
                    if i >= 2:
                        hh_, eb_ = pend.popleft()
                        emit_tail(b, hh_, eb_, catT)
                    if pending_out:
                        pending_out.popleft()()  # prev batch's out-proj
                    for _ in range(3):
                        if fillers:
                            fillers.popleft()()
                while fillers:
                    fillers.popleft()()
                pending_out.extend(mk_outproj(b, catT))
                state.pop(b - 1, None)
            while pending_out:
                pending_out.popleft()()

    nc.compile()
    return nc


_CACHE = {}


def _get_runner():
    """Build the bass module once and a cached jitted SPMD executor."""
    if "run" in _CACHE:
        return _CACHE["run"]

    from jax.sharding import Mesh, PartitionSpec
    from jax.experimental.shard_map import shard_map
    from concourse.bass2jax import (
        _bass_exec_p, install_neuronx_cc_hook, partition_id_tensor)
    import concourse.mybir as mybir_

    nc = bacc.Bacc("TRN2", target_bir_lowering=False, debug=False)
    _emit(nc)

    install_neuronx_cc_hook()

    partition_name = (nc.partition_id_tensor.name
                      if nc.partition_id_tensor else None)
    in_names, out_names, out_avals, zero_outs = [], [], [], []
    for alloc in nc.m.functions[0].allocations:
        if not isinstance(alloc, mybir_.MemoryLocationSet):
            continue
        name = alloc.memorylocations[0].name
        if alloc.kind == "ExternalInput":
            if name != partition_name:
                in_names.append(name)
        elif alloc.kind == "ExternalOutput":
            out_names.append(name)
            shape = tuple(alloc.tensor_shape)
            dtype = mybir_.dt.np(alloc.dtype)
            out_avals.append(jax.core.ShapedArray(shape, dtype))
            zero_outs.append(np.zeros((NCORES * shape[0], *shape[1:]), dtype))
    n_params = len(in_names)
    all_names = in_names + out_names
    if partition_name is not None:
        all_names = all_names + [partition_name]

    def _body(*args):
        operands = list(args)
        if partition_name is not None:
            operands.append(partition_id_tensor())
        outs = _bass_exec_p.bind(
            *operands,
            out_avals=tuple(out_avals),
            in_names=tuple(all_names),
            out_names=tuple(out_names),
            lowering_input_output_aliases=(),
            sim_require_finite=True,
            sim_require_nnan=True,
            nc=nc,
        )
        return tuple(outs)

    devices = jax.devices()[:NCORES]
    mesh = Mesh(np.asarray(devices), ("core",))
    n_outs = len(out_names)
    # x is batch-sharded; weights are replicated (sent once, not 8x)
    spec_of = {n: (PartitionSpec("core") if n == "x" else PartitionSpec())
               for n in in_names}
    sharded = jax.jit(
        shard_map(
            _body, mesh=mesh,
            in_specs=tuple(spec_of[n] for n in in_names)
            + (PartitionSpec("core"),) * n_outs,
            out_specs=(PartitionSpec("core"),) * n_outs,
            check_rep=False,
        ),
        donate_argnums=tuple(range(n_params, n_params + n_outs)),
        keep_unused=True,
    )

    def run(in_map_global):
        args = [in_map_global[n] for n in in_names]
        zeros = [np.zeros_like(z) for z in zero_outs]
        outs = sharded(*args, *zeros)
        return {n: np.asarray(outs[i]) for i, n in enumerate(out_names)}

    def bench(in_map_global, iters=20):
        """Per-call wall time with device-resident inputs (no donation, no
        host transfers in the loop) - includes dispatch + device exec."""
        import time as _t
        from jax.sharding import NamedSharding
        nodonate = jax.jit(
            shard_map(
                _body, mesh=mesh,
                in_specs=tuple(spec_of[n] for n in in_names)
                + (PartitionSpec("core"),) * n_outs,
                out_specs=(PartitionSpec("core"),) * n_outs,
                check_rep=False,
            ),
            keep_unused=True,
        )
        args = [jax.device_put(in_map_global[n], NamedSharding(mesh, spec_of[n]))
                for n in in_names]
        zs = [jax.device_put(z, NamedSharding(mesh, PartitionSpec("core")))
              for z in zero_outs]
        for _ in range(3):
            o = nodonate(*args, *zs)
            jax.block_until_ready(o)
        runs = []
        for _ in range(4):
            t0 = _t.perf_counter()
            for _ in range(iters):
                o = nodonate(*args, *zs)
            jax.block_until_ready(o)
            runs.append((_t.perf_counter() - t0) / iters)
        print("bench pipelined us/iter:",
              " ".join("%.0f" % (r * 1e6) for r in sorted(runs)))
        return min(runs)

    def bench_scan(in_map_global, iters=32):
        """True device time: run the NEFF `iters` times inside one dispatch
        via lax.scan; subtract a 1-iter dispatch to remove fixed overhead."""
        import time as _t
        from jax.sharding import NamedSharding

        def make(n):
            def _bn(*args):
                ins = args[:n_params]

                def body(carry, _):
                    operands = list(ins) + list(carry)
                    if partition_name is not None:
                        operands.append(partition_id_tensor())
                    o = _bass_exec_p.bind(
                        *operands,
                        out_avals=tuple(out_avals),
                        in_names=tuple(all_names),
                        out_names=tuple(out_names),
                        lowering_input_output_aliases=(),
                        sim_require_finite=True,
                        sim_require_nnan=True,
                        nc=nc,
                    )
                    return tuple(o), None

                fin, _ = jax.lax.scan(body, tuple(args[n_params:]), None,
                                      length=n)
                return fin

            return jax.jit(
                shard_map(
                    _bn, mesh=mesh,
                    in_specs=tuple(spec_of[nm] for nm in in_names)
                    + (PartitionSpec("core"),) * n_outs,
                    out_specs=(PartitionSpec("core"),) * n_outs,
                    check_rep=False,
                ),
                keep_unused=True,
            )

        args = [jax.device_put(in_map_global[nm],
                               NamedSharding(mesh, spec_of[nm]))
                for nm in in_names]
        zs = [jax.device_put(z, NamedSharding(mesh, PartitionSpec("core")))
              for z in zero_outs]
        f1, fN = make(1), make(iters)

        def t(f, reps=4):
            best = 1e9
            for _ in range(reps):
                t0 = _t.perf_counter()
                jax.block_until_ready(f(*args, *zs))
                best = min(best, _t.perf_counter() - t0)
            return best

        jax.block_until_ready(f1(*args, *zs))
        jax.block_until_ready(fN(*args, *zs))
        return (t(fN) - t(f1)) / (iters - 1)

    _CACHE["run"] = run
    _CACHE["bench"] = bench
    _CACHE["bench_scan"] = bench_scan
    return run


def kernel(x, Wq, Wk, Wv, Wo, bo):
    run = _get_runner()
    in_map = {
        "x": np.ascontiguousarray(np.asarray(x, np.float32)),      # [128,256,512]
        "Wq": np.asarray(Wq, np.float32),
        "Wk": np.asarray(Wk, np.float32),
        "Wv": np.asarray(Wv, np.float32),
        "Wo": np.asarray(Wo, np.float32),
        "bo": np.asarray(bo, np.float32).reshape(1, D),
    }
    out = run(in_map)["out"]                                       # [128,256,512]
    return out.astype(np.float32)

